# revision 2
# baseline (speedup 1.0000x reference)
"""Multi-head attention ('general' similarity, softmax, out-proj) on 8 trn2
NeuronCores via Bass/Tile.

Sharding: core c handles batch b=c//2, query rows [qh*1024, (qh+1)*1024) with
qh=c%2. Each core computes its own projections (full K/V for its batch), all 8
heads of attention for its query slice, and its slice of the output
projection. Outputs are disjoint -> host just concatenates.

Device layout trick: everything is kept feature-major ("transposed") so every
matmul contraction runs along SBUF partitions:
  Q_l^T[e,q]   = sum_d W_Q[d,e] Q^T[d,q]          (host supplies Q^T)
  K_hw^T[e,s]  = sum_d Wg[d,e] K_l^T[d,s]         (per head, d=e=64)
  scores^T[k,q]= sum_e K_hw^T[e,k] Q_l^T[e,q]     (row-packed: 2 heads share PE)
  P^T          = exp(scores^T)                     (ScalarE, psum->sbuf bf16)
  headaug^T    = sum_k Vaug[k,(v,1)] P^T[k,q]     (65th 'ones' col => rowsums free)
  out[q,o]     = sum_hv (head^T/rowsum)[hv,q] W_m[hv,o]
"""
import sys
import types

import numpy as np
import ml_dtypes

# ---------------------------------------------------------------- axon shim --
# antenv in this image lacks axon_hooks; register the NTFF profiling hook
# ourselves so trace=True works when the caller asks for it.
def _ensure_axon_hooks():
    if 'antenv.axon_hooks' in sys.modules:
        return
    try:
        from trn_agent_boot.trn_boot import _ntff_profile_via_ctypes
        hook = _ntff_profile_via_ctypes('/opt/axon/libaxon_pjrt.so')
    except Exception:
        hook = None
    mod = types.ModuleType('antenv.axon_hooks')
    mod.get_axon_ntff_profile_hook = lambda: hook
    mod.set_axon_ntff_profile_hook = lambda h: None
    sys.modules['antenv.axon_hooks'] = mod


_ensure_axon_hooks()

import concourse.bass as bass
import concourse.mybir as mybir
import concourse.tile as tile
from concourse.bass_utils import run_bass_kernel_spmd

BF16 = mybir.dt.bfloat16
F32 = mybir.dt.float32

P = 128
D = 512          # model dim (= D_K = D_V = D_OUT)
SQ = 1024        # query rows per core
SK = 2048        # key rows (full sequence)
H = 8
DH = 64
NJ = H // 2      # head pairs
NKB = SK // P    # 16 key blocks
NQB = SQ // P    # 8 query blocks
ND = D // P      # 4 feature blocks
EXPF = mybir.ActivationFunctionType.Exp
MULT = mybir.AluOpType.mult


# ------------------------------------------------------- walrus workaround --
# This container's walrus accepts only ONE embedded sync-wait per hw
# instruction. Move all but the last wait of any instruction onto single-wait
# NoOps inserted just before it in the same engine stream.
_SPLIT_CTR = [0]


def _split_multi_waits(nc, max_waits=1):
    def mk_nop(engine, wait):
        _SPLIT_CTR[0] += 1
        nop = mybir.InstNoOp(name=f"antsplitw-{_SPLIT_CTR[0]}", ins=[], outs=[])
        nop.engine = engine
        nop.sync_info = mybir.SyncInfo(on_wait=[wait], on_update=[])
        return nop

    for fn in nc.m.functions:
        for bb in fn.blocks:
            out = []
            changed = False
            for inst in bb.instructions:
                si = inst.sync_info
                waits = list(si.on_wait) if si is not None and si.on_wait else []
                if len(waits) > max_waits:
                    for w in waits[:-max_waits]:
                        out.append(mk_nop(inst.engine, w))
                    si.on_wait = waits[-max_waits:]
                    changed = True
                out.append(inst)
            if changed:
                bb.instructions = out


# ------------------------------------------------------------ device kernel --
def _build_nc():
    nc = bass.Bass("TRN2", target_bir_lowering=False, debug=False)

    qt_d = nc.declare_dram_parameter("qt", [D, SQ], BF16, isOutput=False)
    kt_d = nc.declare_dram_parameter("kt", [D, SK], BF16, isOutput=False)
    vt_d = nc.declare_dram_parameter("vt", [D, SK], BF16, isOutput=False)
    wq_d = nc.declare_dram_parameter("wq", [D, D], BF16, isOutput=False)
    wk_d = nc.declare_dram_parameter("wk", [D, D], BF16, isOutput=False)
    wv_d = nc.declare_dram_parameter("wv", [D, D], BF16, isOutput=False)
    wg2_d = nc.declare_dram_parameter("wg2", [P, DH], BF16, isOutput=False)
    wm_d = nc.declare_dram_parameter("wm", [D, D], BF16, isOutput=False)
    out_d = nc.declare_dram_parameter("out", [SQ, D], F32, isOutput=True)

    with tile.TileContext(nc) as tc:
        with tc.tile_pool(name="cst", bufs=1) as cst, \
             tc.tile_pool(name="pt", bufs=10) as ptp, \
             tc.tile_pool(name="dve", bufs=2) as dvp, \
             tc.tile_pool(name="psS", bufs=2, space="PSUM") as psS, \
             tc.tile_pool(name="psV", bufs=2, space="PSUM") as psV:

            # ---- loads (single big DMA per tensor, 128-partition layout) ----
            qt = cst.tile([P, ND, SQ], BF16, tag="qt")
            nc.sync.dma_start(qt[:], qt_d.rearrange("(k p) q -> p k q", p=P))
            kt = cst.tile([P, ND, SK], BF16, tag="kt")
            nc.sync.dma_start(kt[:], kt_d.rearrange("(k p) s -> p k s", p=P))
            vt = cst.tile([P, ND, SK], BF16, tag="vt")
            nc.sync.dma_start(vt[:], vt_d.rearrange("(k p) s -> p k s", p=P))
            wq = cst.tile([P, ND, D], BF16, tag="wq")
            nc.sync.dma_start(wq[:], wq_d.rearrange("(k p) e -> p k e", p=P))
            wk = cst.tile([P, ND, D], BF16, tag="wk")
            nc.sync.dma_start(wk[:], wk_d.rearrange("(k p) e -> p k e", p=P))
            wv = cst.tile([P, ND, D], BF16, tag="wv")
            nc.sync.dma_start(wv[:], wv_d.rearrange("(k p) e -> p k e", p=P))
            wm = cst.tile([P, ND, D], BF16, tag="wm")
            nc.sync.dma_start(wm[:], wm_d.rearrange("(k p) e -> p k e", p=P))
            wg2 = cst.tile([P, DH], BF16, tag="wg2")
            nc.sync.dma_start(wg2[:], wg2_d[:])

            # fp32 selector for the rowsum-reciprocal partition broadcast
            sel = cst.tile([P, P], F32, tag="sel")
            nc.vector.memset(sel[:], 0.0)
            nc.vector.memset(sel[0:1, 0:DH], 1.0)
            nc.vector.memset(sel[DH:DH + 1, DH:P], 1.0)

            # ---- projections ----
            # Q_l^T (4 x [128, 1024] bf16; rows of tile j = e in [128j,128j+128))
            qlt = [cst.tile([P, SQ], BF16, tag=f"qlt{j}", name=f"qlt{j}") for j in range(NJ)]
            for j in range(NJ):
                ps = psS.tile([P, 1024], F32, tag="psS")
                for qc in range(2):
                    for k in range(ND):
                        nc.tensor.matmul(
                            ps[:, qc * 512:(qc + 1) * 512],
                            wq[:, k, j * P:(j + 1) * P],
                            qt[:, k, qc * 512:(qc + 1) * 512],
                            start=(k == 0), stop=(k == ND - 1))
                nc.vector.tensor_copy(out=qlt[j][:], in_=ps[:])

            # K_l^T (4 x [128, 2048] bf16)
            klt = [cst.tile([P, SK], BF16, tag=f"klt{j}", name=f"klt{j}") for j in range(NJ)]
            for j in range(NJ):
                for half in range(2):
                    ps = psS.tile([P, 1024], F32, tag="psS")
                    for qc in range(2):
                        sc = half * 1024 + qc * 512
                        for k in range(ND):
                            nc.tensor.matmul(
                                ps[:, qc * 512:(qc + 1) * 512],
                                wk[:, k, j * P:(j + 1) * P],
                                kt[:, k, sc:sc + 512],
                                start=(k == 0), stop=(k == ND - 1))
                    nc.vector.tensor_copy(
                        out=klt[j][:, half * 1024:(half + 1) * 1024], in_=ps[:])

            # V_l natural, augmented with a ones column per head:
            # vaug[i][p, h, 0:64] = V_l[128i+p, 64h+v], vaug[i][p, h, 64] = 1
            vaug = [cst.tile([P, H, DH + 1], BF16, tag=f"vaug{i}", name=f"vaug{i}")
                    for i in range(NKB)]
            for i in range(NKB):
                ps = psS.tile([P, 1024], F32, tag="psS")
                for k in range(ND):
                    nc.tensor.matmul(
                        ps[:, 0:512],
                        vt[:, k, i * P:(i + 1) * P],
                        wv[:, k, :],
                        start=(k == 0), stop=(k == ND - 1))
                nc.vector.tensor_copy(
                    out=vaug[i][:, :, 0:DH],
                    in_=ps[:, 0:512].rearrange("p (h v) -> p h v", v=DH))
                nc.vector.memset(vaug[i][:, :, DH:DH + 1], 1.0)

            # K_hw^T per pair: rows 0:64 = head 2j, 64:128 = head 2j+1
            khwt = [cst.tile([P, SK], BF16, tag=f"khwt{j}", name=f"khwt{j}") for j in range(NJ)]
            for j in range(NJ):
                for half in range(2):
                    ps = psS.tile([P, 1024], F32, tag="psS")
                    for qc in range(2):
                        sc = half * 1024 + qc * 512
                        nc.tensor.matmul(
                            ps[0:DH, qc * 512:(qc + 1) * 512],
                            wg2[0:DH, :], klt[j][0:DH, sc:sc + 512],
                            start=True, stop=True)
                        nc.tensor.matmul(
                            ps[DH:P, qc * 512:(qc + 1) * 512],
                            wg2[DH:P, :], klt[j][DH:P, sc:sc + 512],
                            start=True, stop=True, tile_position=(DH, DH))
                    nc.vector.tensor_copy(
                        out=khwt[j][:, half * 1024:(half + 1) * 1024], in_=ps[:])

            # ---- attention per head pair ----
            headt = [cst.tile([P, SQ], BF16, tag=f"headt{j}", name=f"headt{j}") for j in range(NJ)]
            for j in range(NJ):
                pva = psV.tile([DH + 1, SQ], F32, tag="psV")
                pvb = psV.tile([DH + 1, SQ], F32, tag="psV")
                pts = []

                def scores_step(t, j=j, pts=pts):
                    psa = psS.tile([P, SQ], F32, tag="psS")
                    psb = psS.tile([P, SQ], F32, tag="psS")
                    for qc in range(2):
                        s = qc * 512
                        nc.tensor.matmul(
                            psa[:, s:s + 512],
                            khwt[j][0:DH, t * P:(t + 1) * P],
                            qlt[j][0:DH, s:s + 512], start=True, stop=True)
                        nc.tensor.matmul(
                            psb[:, s:s + 512],
                            khwt[j][DH:P, t * P:(t + 1) * P],
                            qlt[j][DH:P, s:s + 512], start=True, stop=True,
                            tile_position=(DH, 0))
                    pta = ptp.tile([P, SQ], BF16, tag="pt")
                    ptb = ptp.tile([P, SQ], BF16, tag="pt")
                    nc.scalar.activation(pta[:], psa[:], EXPF)
                    nc.scalar.activation(ptb[:], psb[:], EXPF)
                    pts.append((pta, ptb))

                def pv_step(t, j=j, pva=pva, pvb=pvb, pts=pts):
                    pta, ptb = pts[t]
                    st, sp = (t == 0), (t == NKB - 1)
                    for qc in range(2):
                        s = qc * 512
                        nc.tensor.matmul(pva[:, s:s + 512],
                                         vaug[t][:, 2 * j, :],
                                         pta[:, s:s + 512], start=st, stop=sp)
                        nc.tensor.matmul(pvb[:, s:s + 512],
                                         vaug[t][:, 2 * j + 1, :],
                                         ptb[:, s:s + 512], start=st, stop=sp)

                # software-pipelined emission: scores run one kblock ahead of PV
                scores_step(0)
                for t in range(1, NKB):
                    scores_step(t)
                    pv_step(t - 1)
                pv_step(NKB - 1)

                # normalize: recw rows 0 / 64 get the two reciprocals, the
                # fp32 selector matmul broadcasts them over 64 partitions each.
                recw = dvp.tile([P, SQ], F32, tag="recw")
                nc.vector.memset(recw[:], 0.0)
                nc.vector.reciprocal(recw[0:1, :], pva[DH:DH + 1, :])
                nc.vector.reciprocal(recw[DH:DH + 1, :], pvb[DH:DH + 1, :])
                rbp = psS.tile([P, SQ], F32, tag="psS")
                for qc in range(2):
                    s = qc * 512
                    nc.tensor.matmul(rbp[:, s:s + 512], sel[:],
                                     recw[:, s:s + 512], start=True, stop=True)
                rbe = dvp.tile([DH, SQ], F32, tag="rbe")
                rbo = dvp.tile([DH, SQ], F32, tag="rbo")
                nc.vector.tensor_copy(out=rbe[:], in_=rbp[0:DH, :])
                nc.vector.tensor_copy(out=rbo[:], in_=rbp[DH:P, :])
                nc.vector.tensor_tensor(headt[j][0:DH, :], pva[0:DH, :],
                                        rbe[:], MULT)
                nc.vector.tensor_tensor(headt[j][DH:P, :], pvb[0:DH, :],
                                        rbo[:], MULT)

            # ---- output projection ----
            for qb in range(NQB):
                ps = psS.tile([P, 1024], F32, tag="psS")
                for j in range(NJ):
                    nc.tensor.matmul(ps[:, 0:512],
                                     headt[j][:, qb * P:(qb + 1) * P],
                                     wm[:, j, :], start=(j == 0),
                                     stop=(j == NJ - 1))
                ot = dvp.tile([P, D], F32, tag="ot")
                nc.vector.tensor_copy(out=ot[:], in_=ps[:, 0:512])
                nc.sync.dma_start(out_d[qb * P:(qb + 1) * P, :], ot[:])

    _split_multi_waits(nc)
    return nc


_NC = None


def _get_nc():
    global _NC
    if _NC is None:
        _NC = _build_nc()
    return _NC


def _prep_in_maps(Q, K, V, W_Q, W_K, W_V, W_gen_S, W_multi_head):
    bf = ml_dtypes.bfloat16
    wq = np.ascontiguousarray(np.asarray(W_Q, np.float32)).astype(bf)
    wk = np.ascontiguousarray(np.asarray(W_K, np.float32)).astype(bf)
    wv = np.ascontiguousarray(np.asarray(W_V, np.float32)).astype(bf)
    wm = np.ascontiguousarray(np.asarray(W_multi_head, np.float32)).astype(bf)
    wg = np.asarray(W_gen_S, np.float32).astype(bf)
    wg2 = np.concatenate([wg, wg], axis=0)  # [128, 64]

    Q = np.asarray(Q, np.float32)
    K = np.asarray(K, np.float32)
    V = np.asarray(V, np.float32)

    in_maps = []
    for c in range(8):
        b, qh = divmod(c, 2)
        qt = np.ascontiguousarray(
            Q[b, qh * SQ:(qh + 1) * SQ, :].T).astype(bf)
        kt = np.ascontiguousarray(K[b].T).astype(bf)
        vt = np.ascontiguousarray(V[b].T).astype(bf)
        in_maps.append({"qt": qt, "kt": kt, "vt": vt, "wq": wq, "wk": wk,
                        "wv": wv, "wg2": wg2, "wm": wm})
    return in_maps


def _run(in_maps, trace=False):
    nc = _get_nc()
    res = run_bass_kernel_spmd(nc, in_maps, list(range(8)), trace=trace)
    out = np.empty((4, SK, D), np.float32)
    for c in range(8):
        b, qh = divmod(c, 2)
        out[b, qh * SQ:(qh + 1) * SQ, :] = res.results[c]["out"]
    return out, res


def kernel(Q, K, V, M, W_Q, W_K, W_V, W_gen_S, W_multi_head):
    in_maps = _prep_in_maps(Q, K, V, W_Q, W_K, W_V, W_gen_S, W_multi_head)
    out, _ = _run(in_maps, trace=False)
    return out


def kernel_traced(Q, K, V, M, W_Q, W_K, W_V, W_gen_S, W_multi_head):
    in_maps = _prep_in_maps(Q, K, V, W_Q, W_K, W_V, W_gen_S, W_multi_head)
    return _run(in_maps, trace=True)


# revision 5
# speedup vs baseline: 1.2639x; 1.2639x over previous
"""Multi-head attention ('general' similarity, softmax, out-proj) on 8 trn2
NeuronCores via Bass/Tile.

Sharding: core c handles batch b=c//2, query rows [qh*1024, (qh+1)*1024) with
qh=c%2. Each core computes its own projections (full K/V for its batch), all 8
heads of attention for its query slice, and its slice of the output
projection. Outputs are disjoint -> host just concatenates.

Device layout trick: everything is kept feature-major ("transposed") so every
matmul contraction runs along SBUF partitions:
  Q_l^T[e,q]   = sum_d W_Q[d,e] Q^T[d,q]          (host supplies Q^T)
  K_hw^T[e,s]  = sum_d Wg[d,e] K_l^T[d,s]         (per head, d=e=64)
  scores^T[k,q]= sum_e K_hw^T[e,k] Q_l^T[e,q]     (row-packed: 2 heads share PE)
  P^T          = exp(scores^T)                     (ScalarE, psum->sbuf bf16)
  headaug^T    = sum_k Vaug[k,(v,1)] P^T[k,q]     (65th 'ones' col => rowsums free)
  out[q,o]     = sum_hv (head^T/rowsum)[hv,q] W_m[hv,o]
"""
import sys
import types

import numpy as np
import ml_dtypes

# ---------------------------------------------------------------- axon shim --
# antenv in this image lacks axon_hooks; register the NTFF profiling hook
# ourselves so trace=True works when the caller asks for it.
def _ensure_axon_hooks():
    if 'antenv.axon_hooks' in sys.modules:
        return
    try:
        from trn_agent_boot.trn_boot import _ntff_profile_via_ctypes
        hook = _ntff_profile_via_ctypes('/opt/axon/libaxon_pjrt.so')
    except Exception:
        hook = None
    mod = types.ModuleType('antenv.axon_hooks')
    mod.get_axon_ntff_profile_hook = lambda: hook
    mod.set_axon_ntff_profile_hook = lambda h: None
    sys.modules['antenv.axon_hooks'] = mod


_ensure_axon_hooks()

import concourse.bass as bass
import concourse.mybir as mybir
import concourse.tile as tile
from concourse.bass_utils import run_bass_kernel_spmd

BF16 = mybir.dt.bfloat16
F32 = mybir.dt.float32

P = 128
D = 512          # model dim (= D_K = D_V = D_OUT)
SQ = 1024        # query rows per core
SK = 2048        # key rows (full sequence)
H = 8
DH = 64
NJ = H // 2      # head pairs
NKB = SK // P    # 16 key blocks
NQB = SQ // P    # 8 query blocks
ND = D // P      # 4 feature blocks
EXPF = mybir.ActivationFunctionType.Exp
MULT = mybir.AluOpType.mult


# ------------------------------------------------------- walrus workaround --
# This container's walrus accepts only ONE embedded sync-wait per hw
# instruction. Move all but the last wait of any instruction onto single-wait
# NoOps inserted just before it in the same engine stream.
_SPLIT_CTR = [0]


def _split_multi_waits(nc, max_waits=1):
    def mk_nop(engine, wait):
        _SPLIT_CTR[0] += 1
        nop = mybir.InstNoOp(name=f"antsplitw-{_SPLIT_CTR[0]}", ins=[], outs=[])
        nop.engine = engine
        nop.sync_info = mybir.SyncInfo(on_wait=[wait], on_update=[])
        return nop

    for fn in nc.m.functions:
        for bb in fn.blocks:
            out = []
            changed = False
            for inst in bb.instructions:
                si = inst.sync_info
                waits = list(si.on_wait) if si is not None and si.on_wait else []
                if len(waits) > max_waits:
                    for w in waits[:-max_waits]:
                        out.append(mk_nop(inst.engine, w))
                    si.on_wait = waits[-max_waits:]
                    changed = True
                out.append(inst)
            if changed:
                bb.instructions = out


# ------------------------------------------------------------ device kernel --
def _build_nc():
    nc = bass.Bass("TRN2", target_bir_lowering=False, debug=False)

    qt_d = nc.declare_dram_parameter("qt", [D, SQ], BF16, isOutput=False)
    kt_d = nc.declare_dram_parameter("kt", [D, SK], BF16, isOutput=False)
    vt_d = nc.declare_dram_parameter("vt", [D, SK], BF16, isOutput=False)
    wq_d = nc.declare_dram_parameter("wq", [D, D], BF16, isOutput=False)
    wk_d = nc.declare_dram_parameter("wk", [D, D], BF16, isOutput=False)
    wv_d = nc.declare_dram_parameter("wv", [D, D], BF16, isOutput=False)
    wg2_d = nc.declare_dram_parameter("wg2", [P, DH], BF16, isOutput=False)
    wm_d = nc.declare_dram_parameter("wm", [D, D], BF16, isOutput=False)
    out_d = nc.declare_dram_parameter("out", [SQ, D], F32, isOutput=True)

    with tile.TileContext(nc) as tc:
        with tc.tile_pool(name="cst", bufs=1) as cst, \
             tc.tile_pool(name="pt", bufs=10) as ptp, \
             tc.tile_pool(name="dve", bufs=2) as dvp, \
             tc.tile_pool(name="psS", bufs=2, space="PSUM") as psS, \
             tc.tile_pool(name="psV", bufs=2, space="PSUM") as psV:

            # ---- loads (single big DMA per tensor, 128-partition layout) ----
            # ordered so the Q projection's operands land first
            wq = cst.tile([P, ND, D], BF16, tag="wq")
            nc.sync.dma_start(wq[:], wq_d.rearrange("(k p) e -> p k e", p=P))
            qt = cst.tile([P, ND, SQ], BF16, tag="qt")
            nc.sync.dma_start(qt[:], qt_d.rearrange("(k p) q -> p k q", p=P))
            wk = cst.tile([P, ND, D], BF16, tag="wk")
            nc.sync.dma_start(wk[:], wk_d.rearrange("(k p) e -> p k e", p=P))
            kt = cst.tile([P, ND, SK], BF16, tag="kt")
            nc.sync.dma_start(kt[:], kt_d.rearrange("(k p) s -> p k s", p=P))
            wg2 = cst.tile([P, DH], BF16, tag="wg2")
            nc.sync.dma_start(wg2[:], wg2_d[:])
            wv = cst.tile([P, ND, D], BF16, tag="wv")
            nc.sync.dma_start(wv[:], wv_d.rearrange("(k p) e -> p k e", p=P))
            vt = cst.tile([P, ND, SK], BF16, tag="vt")
            nc.sync.dma_start(vt[:], vt_d.rearrange("(k p) s -> p k s", p=P))
            wm = cst.tile([P, ND, D], BF16, tag="wm")
            nc.sync.dma_start(wm[:], wm_d.rearrange("(k p) e -> p k e", p=P))

            # fp32 selector for the rowsum-reciprocal partition broadcast:
            # row 0 -> output partitions 0:64, row 32 -> partitions 64:128
            sel = cst.tile([DH, P], F32, tag="sel")
            nc.vector.memset(sel[:], 0.0)
            nc.vector.memset(sel[0:1, 0:DH], 1.0)
            nc.vector.memset(sel[32:33, DH:P], 1.0)

            # ---- projections ----
            # Q_l^T (4 x [128, 1024] bf16; rows of tile j = e in [128j,128j+128))
            qlt = [cst.tile([P, SQ], BF16, tag=f"qlt{j}", name=f"qlt{j}") for j in range(NJ)]
            for j in range(NJ):
                ps = psS.tile([P, 1024], F32, tag="psS")
                for qc in range(2):
                    for k in range(ND):
                        nc.tensor.matmul(
                            ps[:, qc * 512:(qc + 1) * 512],
                            wq[:, k, j * P:(j + 1) * P],
                            qt[:, k, qc * 512:(qc + 1) * 512],
                            start=(k == 0), stop=(k == ND - 1))
                nc.vector.tensor_copy(out=qlt[j][:], in_=ps[:])

            # K_l^T (4 x [128, 2048] bf16)
            klt = [cst.tile([P, SK], BF16, tag=f"klt{j}", name=f"klt{j}") for j in range(NJ)]
            for j in range(NJ):
                for half in range(2):
                    ps = psS.tile([P, 1024], F32, tag="psS")
                    for qc in range(2):
                        sc = half * 1024 + qc * 512
                        for k in range(ND):
                            nc.tensor.matmul(
                                ps[:, qc * 512:(qc + 1) * 512],
                                wk[:, k, j * P:(j + 1) * P],
                                kt[:, k, sc:sc + 512],
                                start=(k == 0), stop=(k == ND - 1))
                    nc.vector.tensor_copy(
                        out=klt[j][:, half * 1024:(half + 1) * 1024], in_=ps[:])

            # V_l natural, augmented with a ones column per head:
            # vaug[i][p, h, 0:64] = V_l[128i+p, 64h+v], vaug[i][p, h, 64] = 1
            vaug = [cst.tile([P, H, DH + 1], BF16, tag=f"vaug{i}", name=f"vaug{i}")
                    for i in range(NKB)]
            for i in range(NKB):
                ps = psS.tile([P, 1024], F32, tag="psS")
                for k in range(ND):
                    nc.tensor.matmul(
                        ps[:, 0:512],
                        vt[:, k, i * P:(i + 1) * P],
                        wv[:, k, :],
                        start=(k == 0), stop=(k == ND - 1))
                nc.vector.tensor_copy(
                    out=vaug[i][:, :, 0:DH],
                    in_=ps[:, 0:512].rearrange("p (h v) -> p h v", v=DH))
                nc.vector.memset(vaug[i][:, :, DH:DH + 1], 1.0)

            # K_hw^T per pair: rows 0:64 = head 2j, 64:128 = head 2j+1
            khwt = [cst.tile([P, SK], BF16, tag=f"khwt{j}", name=f"khwt{j}") for j in range(NJ)]
            for j in range(NJ):
                for half in range(2):
                    ps = psS.tile([P, 1024], F32, tag="psS")
                    for qc in range(2):
                        sc = half * 1024 + qc * 512
                        nc.tensor.matmul(
                            ps[0:DH, qc * 512:(qc + 1) * 512],
                            wg2[0:DH, :], klt[j][0:DH, sc:sc + 512],
                            start=True, stop=True)
                        nc.tensor.matmul(
                            ps[DH:P, qc * 512:(qc + 1) * 512],
                            wg2[DH:P, :], klt[j][DH:P, sc:sc + 512],
                            start=True, stop=True, tile_position=(DH, DH))
                    nc.vector.tensor_copy(
                        out=khwt[j][:, half * 1024:(half + 1) * 1024], in_=ps[:])

            # ---- attention per head pair ----
            headt = [cst.tile([P, SQ], BF16, tag=f"headt{j}", name=f"headt{j}") for j in range(NJ)]
            LOGF = mybir.ActivationFunctionType.Ln
            pend = []  # deferred normalize tails (run under next pair's scores)

            for j in range(NJ):
                pva = psV.tile([DH + 1, SQ], F32, tag="psV", name="pva")
                pvb = psV.tile([DH + 1, SQ], F32, tag="psV", name="pvb")
                pts = []

                def scores_step(t, j=j, pts=pts):
                    psa = psS.tile([P, SQ], F32, tag="psS", name="psa")
                    psb = psS.tile([P, SQ], F32, tag="psS", name="psb")
                    for qc in range(2):
                        s = qc * 512
                        nc.tensor.matmul(
                            psa[:, s:s + 512],
                            khwt[j][0:DH, t * P:(t + 1) * P],
                            qlt[j][0:DH, s:s + 512], start=True, stop=True)
                        nc.tensor.matmul(
                            psb[:, s:s + 512],
                            khwt[j][DH:P, t * P:(t + 1) * P],
                            qlt[j][DH:P, s:s + 512], start=True, stop=True,
                            tile_position=(DH, 0))
                    pta = ptp.tile([P, SQ], BF16, tag="pt", name="pta")
                    ptb = ptp.tile([P, SQ], BF16, tag="pt", name="ptb")
                    nc.scalar.activation(pta[:], psa[:], EXPF)
                    nc.scalar.activation(ptb[:], psb[:], EXPF)
                    pts.append((pta, ptb))

                def pv_step(t, j=j, pva=pva, pvb=pvb, pts=pts):
                    pta, ptb = pts[t]
                    st, sp = (t == 0), (t == NKB - 1)
                    for qc in range(2):
                        s = qc * 512
                        nc.tensor.matmul(pva[:, s:s + 512],
                                         vaug[t][:, 2 * j, :],
                                         pta[:, s:s + 512], start=st, stop=sp)
                        nc.tensor.matmul(pvb[:, s:s + 512],
                                         vaug[t][:, 2 * j + 1, :],
                                         ptb[:, s:s + 512], start=st, stop=sp)

                # software-pipelined emission: scores run one kblock ahead of
                # PV; the previous pair's normalize tail slots in after two
                # score steps so its rowsum reciprocals are ready by then.
                scores_step(0)
                scores_step(1)
                if pend:
                    pend.pop()()
                pv_step(0)
                for t in range(2, NKB):
                    scores_step(t)
                    pv_step(t - 1)
                pv_step(NKB - 1)

                # normalize part 1: gather the two rowsum rows, then
                # 1/x = exp(-log(x)) on ScalarE (DVE reciprocal is ~6x slower)
                recw = dvp.tile([DH, SQ], F32, tag="recw", name="recw")
                nc.vector.memset(recw[:], 1.0)
                nc.vector.tensor_copy(out=recw[0:1, :], in_=pva[DH:DH + 1, :])
                nc.vector.tensor_copy(out=recw[32:33, :], in_=pvb[DH:DH + 1, :])
                lg = dvp.tile([DH, SQ], F32, tag="lg", name="lg")
                nc.scalar.activation(lg[:], recw[:], LOGF)
                recr = dvp.tile([DH, SQ], F32, tag="recr", name="recr")
                nc.scalar.activation(recr[:], lg[:], EXPF, scale=-1.0)

                def part2(j=j, pva=pva, pvb=pvb, recr=recr):
                    rbp = psS.tile([P, SQ], F32, tag="psS", name="rbp")
                    for qc in range(2):
                        s = qc * 512
                        nc.tensor.matmul(rbp[:, s:s + 512], sel[:],
                                         recr[:, s:s + 512], start=True,
                                         stop=True)
                    rbe = dvp.tile([DH, SQ], F32, tag="rbe", name="rbe")
                    rbo = dvp.tile([DH, SQ], F32, tag="rbo", name="rbo")
                    nc.vector.tensor_copy(out=rbe[:], in_=rbp[0:DH, :])
                    nc.vector.tensor_copy(out=rbo[:], in_=rbp[DH:P, :])
                    nc.vector.tensor_tensor(headt[j][0:DH, :], pva[0:DH, :],
                                            rbe[:], MULT)
                    nc.vector.tensor_tensor(headt[j][DH:P, :], pvb[0:DH, :],
                                            rbo[:], MULT)

                pend.append(part2)

            while pend:
                pend.pop()()

            # ---- output projection ----
            for qb in range(NQB):
                ps = psS.tile([P, 1024], F32, tag="psS")
                for j in range(NJ):
                    nc.tensor.matmul(ps[:, 0:512],
                                     headt[j][:, qb * P:(qb + 1) * P],
                                     wm[:, j, :], start=(j == 0),
                                     stop=(j == NJ - 1))
                ot = dvp.tile([P, D], F32, tag="ot")
                nc.vector.tensor_copy(out=ot[:], in_=ps[:, 0:512])
                nc.sync.dma_start(out_d[qb * P:(qb + 1) * P, :], ot[:])

    _split_multi_waits(nc)
    return nc


_NC = None


def _get_nc():
    global _NC
    if _NC is None:
        _NC = _build_nc()
    return _NC


def _prep_in_maps(Q, K, V, W_Q, W_K, W_V, W_gen_S, W_multi_head):
    bf = ml_dtypes.bfloat16
    wq = np.ascontiguousarray(np.asarray(W_Q, np.float32)).astype(bf)
    wk = np.ascontiguousarray(np.asarray(W_K, np.float32)).astype(bf)
    wv = np.ascontiguousarray(np.asarray(W_V, np.float32)).astype(bf)
    wm = np.ascontiguousarray(np.asarray(W_multi_head, np.float32)).astype(bf)
    wg = np.asarray(W_gen_S, np.float32).astype(bf)
    wg2 = np.concatenate([wg, wg], axis=0)  # [128, 64]

    Q = np.asarray(Q, np.float32)
    K = np.asarray(K, np.float32)
    V = np.asarray(V, np.float32)

    in_maps = []
    for c in range(8):
        b, qh = divmod(c, 2)
        qt = np.ascontiguousarray(
            Q[b, qh * SQ:(qh + 1) * SQ, :].T).astype(bf)
        kt = np.ascontiguousarray(K[b].T).astype(bf)
        vt = np.ascontiguousarray(V[b].T).astype(bf)
        in_maps.append({"qt": qt, "kt": kt, "vt": vt, "wq": wq, "wk": wk,
                        "wv": wv, "wg2": wg2, "wm": wm})
    return in_maps


def _run(in_maps, trace=False):
    nc = _get_nc()
    res = run_bass_kernel_spmd(nc, in_maps, list(range(8)), trace=trace)
    out = np.empty((4, SK, D), np.float32)
    for c in range(8):
        b, qh = divmod(c, 2)
        out[b, qh * SQ:(qh + 1) * SQ, :] = res.results[c]["out"]
    return out, res


def kernel(Q, K, V, M, W_Q, W_K, W_V, W_gen_S, W_multi_head):
    in_maps = _prep_in_maps(Q, K, V, W_Q, W_K, W_V, W_gen_S, W_multi_head)
    out, _ = _run(in_maps, trace=False)
    return out


def kernel_traced(Q, K, V, M, W_Q, W_K, W_V, W_gen_S, W_multi_head):
    in_maps = _prep_in_maps(Q, K, V, W_Q, W_K, W_V, W_gen_S, W_multi_head)
    return _run(in_maps, trace=True)


# revision 9
# speedup vs baseline: 1.4065x; 1.1128x over previous
"""Multi-head attention ('general' similarity, softmax, out-proj) on 8 trn2
NeuronCores via Bass/Tile.

Sharding: core c handles batch b=c//2, query rows [qh*1024, (qh+1)*1024) with
qh=c%2. Each core computes its own projections (full K/V for its batch), all 8
heads of attention for its query slice, and its slice of the output
projection. Outputs are disjoint -> host just concatenates.

Device layout trick: everything is kept feature-major ("transposed") so every
matmul contraction runs along SBUF partitions:
  Q_l^T[e,q]   = sum_d W_Q[d,e] Q^T[d,q]          (host supplies Q^T)
  K_hw^T[e,s]  = sum_d Wg[d,e] K_l^T[d,s]         (per head, d=e=64)
  scores^T[k,q]= sum_e K_hw^T[e,k] Q_l^T[e,q]     (row-packed: 2 heads share PE)
  P^T          = exp(scores^T)                     (ScalarE, psum->sbuf bf16)
  headaug^T    = sum_k Vaug[k,(v,1)] P^T[k,q]     (65th 'ones' col => rowsums free)
  out[q,o]     = sum_hv (head^T/rowsum)[hv,q] W_m[hv,o]
"""
import sys
import types

import numpy as np
import ml_dtypes

# ---------------------------------------------------------------- axon shim --
# antenv in this image lacks axon_hooks; register the NTFF profiling hook
# ourselves so trace=True works when the caller asks for it.
def _ensure_axon_hooks():
    if 'antenv.axon_hooks' in sys.modules:
        return
    try:
        from trn_agent_boot.trn_boot import _ntff_profile_via_ctypes
        hook = _ntff_profile_via_ctypes('/opt/axon/libaxon_pjrt.so')
    except Exception:
        hook = None
    mod = types.ModuleType('antenv.axon_hooks')
    mod.get_axon_ntff_profile_hook = lambda: hook
    mod.set_axon_ntff_profile_hook = lambda h: None
    sys.modules['antenv.axon_hooks'] = mod


_ensure_axon_hooks()

import concourse.bass as bass
import concourse.mybir as mybir
import concourse.tile as tile
from concourse.bass_utils import run_bass_kernel_spmd

BF16 = mybir.dt.bfloat16
F32 = mybir.dt.float32

P = 128
D = 512          # model dim (= D_K = D_V = D_OUT)
SQ = 1024        # query rows per core
SK = 2048        # key rows (full sequence)
H = 8
DH = 64
NJ = H // 2      # head pairs
NKB = SK // P    # 16 key blocks
NQB = SQ // P    # 8 query blocks
ND = D // P      # 4 feature blocks
EXPF = mybir.ActivationFunctionType.Exp
MULT = mybir.AluOpType.mult


# ------------------------------------------------------- walrus workaround --
# This container's walrus accepts only ONE embedded sync-wait per hw
# instruction. Move all but the last wait of any instruction onto single-wait
# NoOps inserted just before it in the same engine stream.
_SPLIT_CTR = [0]


def _split_multi_waits(nc, max_waits=1):
    def mk_nop(engine, wait):
        _SPLIT_CTR[0] += 1
        nop = mybir.InstNoOp(name=f"antsplitw-{_SPLIT_CTR[0]}", ins=[], outs=[])
        nop.engine = engine
        nop.sync_info = mybir.SyncInfo(on_wait=[wait], on_update=[])
        return nop

    for fn in nc.m.functions:
        for bb in fn.blocks:
            out = []
            changed = False
            for inst in bb.instructions:
                si = inst.sync_info
                waits = list(si.on_wait) if si is not None and si.on_wait else []
                if len(waits) > max_waits:
                    for w in waits[:-max_waits]:
                        out.append(mk_nop(inst.engine, w))
                    si.on_wait = waits[-max_waits:]
                    changed = True
                out.append(inst)
            if changed:
                bb.instructions = out


# ------------------------------------------------------------ device kernel --
def _build_nc():
    nc = bass.Bass("TRN2", target_bir_lowering=False, debug=False)

    qt_d = nc.declare_dram_parameter("qt", [D, SQ], BF16, isOutput=False)
    kt_d = nc.declare_dram_parameter("kt", [D, SK], BF16, isOutput=False)
    vt_d = nc.declare_dram_parameter("vt", [D, SK], BF16, isOutput=False)
    wq_d = nc.declare_dram_parameter("wq", [D, D], BF16, isOutput=False)
    wk_d = nc.declare_dram_parameter("wk", [D, D], BF16, isOutput=False)
    wv_d = nc.declare_dram_parameter("wv", [D, D], BF16, isOutput=False)
    wg2_d = nc.declare_dram_parameter("wg2", [P, DH], BF16, isOutput=False)
    wm_d = nc.declare_dram_parameter("wm", [D, D], BF16, isOutput=False)
    out_d = nc.declare_dram_parameter("out", [SQ, D], F32, isOutput=True)

    with tile.TileContext(nc) as tc:
        with tc.tile_pool(name="cst", bufs=1) as cst, \
             tc.tile_pool(name="pt", bufs=16) as ptp, \
             tc.tile_pool(name="dve", bufs=2) as dvp, \
             tc.tile_pool(name="psS", bufs=2, space="PSUM") as psS, \
             tc.tile_pool(name="psV", bufs=2, space="PSUM") as psV:

            # ---- loads (single big DMA per tensor, 128-partition layout) ----
            # ordered so the Q projection's operands land first
            wq = cst.tile([P, ND, D], BF16, tag="wq")
            nc.sync.dma_start(wq[:], wq_d.rearrange("(k p) e -> p k e", p=P))
            qt = cst.tile([P, ND, SQ], BF16, tag="qt")
            nc.sync.dma_start(qt[:], qt_d.rearrange("(k p) q -> p k q", p=P))
            wk = cst.tile([P, ND, D], BF16, tag="wk")
            nc.sync.dma_start(wk[:], wk_d.rearrange("(k p) e -> p k e", p=P))
            kt = cst.tile([P, ND, SK], BF16, tag="kt")
            nc.sync.dma_start(kt[:], kt_d.rearrange("(k p) s -> p k s", p=P))
            wg2 = cst.tile([P, DH], BF16, tag="wg2")
            nc.sync.dma_start(wg2[:], wg2_d[:])
            wv = cst.tile([P, ND, D], BF16, tag="wv")
            nc.sync.dma_start(wv[:], wv_d.rearrange("(k p) e -> p k e", p=P))
            vt = cst.tile([P, ND, SK], BF16, tag="vt")
            nc.sync.dma_start(vt[:], vt_d.rearrange("(k p) s -> p k s", p=P))
            wm = cst.tile([P, ND, D], BF16, tag="wm")
            nc.sync.dma_start(wm[:], wm_d.rearrange("(k p) e -> p k e", p=P))

            # fp32 selector for the rowsum-reciprocal partition broadcast:
            # row 0 -> output partitions 0:64, row 32 -> partitions 64:128
            sel = cst.tile([DH, P], F32, tag="sel")
            nc.vector.memset(sel[:], 0.0)
            nc.vector.memset(sel[0:1, 0:DH], 1.0)
            nc.vector.memset(sel[32:33, DH:P], 1.0)

            # ---- projections (emitted as chunks, interleaved below) ----
            qlt = [cst.tile([P, SQ], BF16, tag=f"qlt{j}", name=f"qlt{j}") for j in range(NJ)]
            klt = [cst.tile([P, SK], BF16, tag=f"klt{j}", name=f"klt{j}") for j in range(NJ)]
            vaug = [cst.tile([P, H, DH + 1], BF16, tag=f"vaug{i}", name=f"vaug{i}")
                    for i in range(NKB)]
            khwt = [cst.tile([P, SK], BF16, tag=f"khwt{j}", name=f"khwt{j}") for j in range(NJ)]

            def qproj(j):
                # Q_l^T tile j (rows = e in [128j, 128j+128))
                ps = psS.tile([P, 1024], F32, tag="psS", name="psq")
                for qc in range(2):
                    for k in range(ND):
                        nc.tensor.matmul(
                            ps[:, qc * 512:(qc + 1) * 512],
                            wq[:, k, j * P:(j + 1) * P],
                            qt[:, k, qc * 512:(qc + 1) * 512],
                            start=(k == 0), stop=(k == ND - 1))
                nc.vector.tensor_copy(out=qlt[j][:], in_=ps[:])

            def kproj(j, half):
                ps = psS.tile([P, 1024], F32, tag="psS", name="psk")
                for qc in range(2):
                    sc = half * 1024 + qc * 512
                    for k in range(ND):
                        nc.tensor.matmul(
                            ps[:, qc * 512:(qc + 1) * 512],
                            wk[:, k, j * P:(j + 1) * P],
                            kt[:, k, sc:sc + 512],
                            start=(k == 0), stop=(k == ND - 1))
                nc.vector.tensor_copy(
                    out=klt[j][:, half * 1024:(half + 1) * 1024], in_=ps[:])

            def vproj(i):
                # V_l rows [128i, 128i+128), all heads + the ones column
                ps = psS.tile([P, 1024], F32, tag="psS", name="psv")
                for k in range(ND):
                    nc.tensor.matmul(
                        ps[:, 0:512],
                        vt[:, k, i * P:(i + 1) * P],
                        wv[:, k, :],
                        start=(k == 0), stop=(k == ND - 1))
                nc.vector.tensor_copy(
                    out=vaug[i][:, :, 0:DH],
                    in_=ps[:, 0:512].rearrange("p (h v) -> p h v", v=DH))
                nc.vector.memset(vaug[i][:, :, DH:DH + 1], 1.0)

            def khw(j, half):
                # K_hw^T rows 0:64 = head 2j, 64:128 = head 2j+1
                ps = psS.tile([P, 1024], F32, tag="psS", name="psh")
                for qc in range(2):
                    sc = half * 1024 + qc * 512
                    nc.tensor.matmul(
                        ps[0:DH, qc * 512:(qc + 1) * 512],
                        wg2[0:DH, :], klt[j][0:DH, sc:sc + 512],
                        start=True, stop=True)
                    nc.tensor.matmul(
                        ps[DH:P, qc * 512:(qc + 1) * 512],
                        wg2[DH:P, :], klt[j][DH:P, sc:sc + 512],
                        start=True, stop=True, tile_position=(DH, DH))
                nc.vector.tensor_copy(
                    out=khwt[j][:, half * 1024:(half + 1) * 1024], in_=ps[:])

            # prologue: only what pair 0 + pair 1 scores need; everything else
            # interleaves into the attention t-loops to keep ScalarE saturated
            for j in (0, 1):
                qproj(j)
                kproj(j, 0)
                kproj(j, 1)
                khw(j, 0)
                khw(j, 1)

            # per-pair extra work to interleave, one chunk per t step
            extras = {
                0: [(lambda i=i: vproj(i)) for i in range(NKB)],
                1: [lambda: qproj(2), lambda: kproj(2, 0), lambda: kproj(2, 1),
                    lambda: khw(2, 0), lambda: khw(2, 1)],
                2: [lambda: qproj(3), lambda: kproj(3, 0), lambda: kproj(3, 1),
                    lambda: khw(3, 0), lambda: khw(3, 1)],
                3: [],
            }

            # ---- attention per head pair ----
            headt = [cst.tile([P, SQ], BF16, tag=f"headt{j}", name=f"headt{j}") for j in range(NJ)]
            LOGF = mybir.ActivationFunctionType.Ln
            pend = []  # deferred normalize tails (run under next pair's scores)

            for j in range(NJ):
                pva = psV.tile([DH + 1, SQ], F32, tag="psV", name="pva")
                pvb = psV.tile([DH + 1, SQ], F32, tag="psV", name="pvb")
                pts = []

                def scores_step(t, j=j, pts=pts):
                    psa = psS.tile([P, SQ], F32, tag="psS", name="psa")
                    psb = psS.tile([P, SQ], F32, tag="psS", name="psb")
                    for qc in range(2):
                        s = qc * 512
                        nc.tensor.matmul(
                            psa[:, s:s + 512],
                            khwt[j][0:DH, t * P:(t + 1) * P],
                            qlt[j][0:DH, s:s + 512], start=True, stop=True)
                        nc.tensor.matmul(
                            psb[:, s:s + 512],
                            khwt[j][DH:P, t * P:(t + 1) * P],
                            qlt[j][DH:P, s:s + 512], start=True, stop=True,
                            tile_position=(DH, 0))
                    pta = ptp.tile([P, SQ], BF16, tag="pt", name="pta")
                    ptb = ptp.tile([P, SQ], BF16, tag="pt", name="ptb")
                    nc.scalar.activation(pta[:], psa[:], EXPF)
                    nc.scalar.activation(ptb[:], psb[:], EXPF)
                    pts.append((pta, ptb))

                def pv_step(t, j=j, pva=pva, pvb=pvb, pts=pts):
                    pta, ptb = pts[t]
                    st, sp = (t == 0), (t == NKB - 1)
                    for qc in range(2):
                        s = qc * 512
                        nc.tensor.matmul(pva[:, s:s + 512],
                                         vaug[t][:, 2 * j, :],
                                         pta[:, s:s + 512], start=st, stop=sp)
                        nc.tensor.matmul(pvb[:, s:s + 512],
                                         vaug[t][:, 2 * j + 1, :],
                                         ptb[:, s:s + 512], start=st, stop=sp)

                # software-pipelined emission: scores run one kblock ahead of
                # PV; the previous pair's normalize tail slots in after two
                # score steps so its rowsum reciprocals are ready by then.
                ext = extras[j]
                scores_step(0)
                if ext:
                    ext.pop(0)()
                scores_step(1)
                if ext:
                    ext.pop(0)()
                if pend:
                    pend.pop()()
                pv_step(0)
                for t in range(2, NKB):
                    scores_step(t)
                    if ext:
                        ext.pop(0)()
                    pv_step(t - 1)
                while ext:
                    ext.pop(0)()
                pv_step(NKB - 1)

                # normalize part 1: gather the two rowsum rows, then
                # 1/x = exp(-log(x)) on ScalarE (DVE reciprocal is ~6x slower)
                recw = dvp.tile([DH, SQ], F32, tag="recw", name="recw")
                nc.vector.memset(recw[:], 1.0)
                nc.vector.tensor_copy(out=recw[0:1, :], in_=pva[DH:DH + 1, :])
                nc.vector.tensor_copy(out=recw[32:33, :], in_=pvb[DH:DH + 1, :])
                lg = dvp.tile([DH, SQ], F32, tag="lg", name="lg")
                nc.scalar.activation(lg[:], recw[:], LOGF)
                recr = dvp.tile([DH, SQ], F32, tag="recr", name="recr")
                nc.scalar.activation(recr[:], lg[:], EXPF, scale=-1.0)

                def part2(j=j, pva=pva, pvb=pvb, recr=recr):
                    rbp = psS.tile([P, SQ], F32, tag="psS", name="rbp")
                    for qc in range(2):
                        s = qc * 512
                        nc.tensor.matmul(rbp[:, s:s + 512], sel[:],
                                         recr[:, s:s + 512], start=True,
                                         stop=True)
                    rbe = dvp.tile([DH, SQ], F32, tag="rbe", name="rbe")
                    rbo = dvp.tile([DH, SQ], F32, tag="rbo", name="rbo")
                    nc.vector.tensor_copy(out=rbe[:], in_=rbp[0:DH, :])
                    nc.vector.tensor_copy(out=rbo[:], in_=rbp[DH:P, :])
                    nc.vector.tensor_tensor(headt[j][0:DH, :], pva[0:DH, :],
                                            rbe[:], MULT)
                    nc.vector.tensor_tensor(headt[j][DH:P, :], pvb[0:DH, :],
                                            rbo[:], MULT)

                pend.append(part2)

            while pend:
                pend.pop()()

            # ---- output projection ----
            for qb in range(NQB):
                ps = psS.tile([P, 1024], F32, tag="psS")
                for j in range(NJ):
                    nc.tensor.matmul(ps[:, 0:512],
                                     headt[j][:, qb * P:(qb + 1) * P],
                                     wm[:, j, :], start=(j == 0),
                                     stop=(j == NJ - 1))
                ot = dvp.tile([P, D], F32, tag="ot")
                nc.vector.tensor_copy(out=ot[:], in_=ps[:, 0:512])
                nc.sync.dma_start(out_d[qb * P:(qb + 1) * P, :], ot[:])

    _split_multi_waits(nc)
    return nc


_NC = None


def _get_nc():
    global _NC
    if _NC is None:
        _NC = _build_nc()
    return _NC


def _prep_in_maps(Q, K, V, W_Q, W_K, W_V, W_gen_S, W_multi_head):
    bf = ml_dtypes.bfloat16
    wq = np.ascontiguousarray(np.asarray(W_Q, np.float32)).astype(bf)
    wk = np.ascontiguousarray(np.asarray(W_K, np.float32)).astype(bf)
    wv = np.ascontiguousarray(np.asarray(W_V, np.float32)).astype(bf)
    wm = np.ascontiguousarray(np.asarray(W_multi_head, np.float32)).astype(bf)
    wg = np.asarray(W_gen_S, np.float32).astype(bf)
    wg2 = np.concatenate([wg, wg], axis=0)  # [128, 64]

    Q = np.asarray(Q, np.float32)
    K = np.asarray(K, np.float32)
    V = np.asarray(V, np.float32)

    in_maps = []
    for c in range(8):
        b, qh = divmod(c, 2)
        qt = np.ascontiguousarray(
            Q[b, qh * SQ:(qh + 1) * SQ, :].T).astype(bf)
        kt = np.ascontiguousarray(K[b].T).astype(bf)
        vt = np.ascontiguousarray(V[b].T).astype(bf)
        in_maps.append({"qt": qt, "kt": kt, "vt": vt, "wq": wq, "wk": wk,
                        "wv": wv, "wg2": wg2, "wm": wm})
    return in_maps


def _run(in_maps, trace=False):
    nc = _get_nc()
    res = run_bass_kernel_spmd(nc, in_maps, list(range(8)), trace=trace)
    out = np.empty((4, SK, D), np.float32)
    for c in range(8):
        b, qh = divmod(c, 2)
        out[b, qh * SQ:(qh + 1) * SQ, :] = res.results[c]["out"]
    return out, res


def kernel(Q, K, V, M, W_Q, W_K, W_V, W_gen_S, W_multi_head):
    in_maps = _prep_in_maps(Q, K, V, W_Q, W_K, W_V, W_gen_S, W_multi_head)
    out, _ = _run(in_maps, trace=False)
    return out


def kernel_traced(Q, K, V, M, W_Q, W_K, W_V, W_gen_S, W_multi_head):
    in_maps = _prep_in_maps(Q, K, V, W_Q, W_K, W_V, W_gen_S, W_multi_head)
    return _run(in_maps, trace=True)
